# revision 14
# baseline (speedup 1.0000x reference)
# Trainium2 Bass kernel for nn_CustomConv2D_57200374448719:
#   data [32,128,64,64] f32 (NCHW) conv weights [256,128,3,3] (OIHW),
#   VALID, stride 1 -> out [32,256,62,62] f32.
#
# Strategy: data-parallel over batch across 8 NeuronCores (4 images per
# core), weights replicated. Per core, implicit GEMM with C_in=128 on the
# SBUF partition axis: for each image / C_out half (128) / group of 8
# output rows, accumulate 9 matmuls (one per 3x3 tap, K=128, N=rows*62)
# into one PSUM bank. The shifted conv windows are strided access
# patterns on the resident image tiles, so no im2col copy is ever
# materialized. Matmuls run in float32r (bit-identical fp32 in memory,
# reduced-precision multiply at full PE rate; measured error identical to
# the hardware fp32 path); accumulation is fp32 in PSUM.
#
# Startup-latency hiding: weights are loaded as two per-co-half chunks
# and each image as two halo'd row-halves, all on the sync-engine HWDGE
# ring, ordered so the first row-group's dependencies (first weight half
# + first image half) land as early as possible. Output stores go on the
# scalar-engine HWDGE ring (separate FIFO) per row-group so they stream
# out during compute.
import numpy as np

N_CORES = 8
B, CIN, H, W = 32, 128, 64, 64
COUT, KH, KW = 256, 3, 3
OH, OW = H - KH + 1, W - KW + 1  # 62, 62
BPC = B // N_CORES  # images per core
ROW_GROUPS = [(r0, min(8, OH - r0)) for r0 in range(0, OH, 8)]  # 7x8 + 1x6
# image row chunks (with conv halo): A1 rows [0,18) serves row-groups 0-1,
# A2 rows [16,34) serves 2-3, B rows [32,64) serves 4-7
CHUNKS = [(0, 18), (16, 18), (32, 32)]

_cache = {}


def build_nc(mm_dtype_name="float32r"):
    import concourse.bacc as bacc
    import concourse.mybir as mybir
    import concourse.tile as tile

    mm_dt = getattr(mybir.dt, mm_dtype_name)
    f32 = mybir.dt.float32

    nc = bacc.Bacc("TRN2", target_bir_lowering=False, debug=False, num_devices=N_CORES)
    data_in = nc.dram_tensor("data", [BPC, CIN, H, W], mm_dt, kind="ExternalInput").ap()
    # wt[ci, g*(9*128) + t*128 + co'] = weights[g*128+co', ci, ky, kx], t=ky*3+kx
    w_in = nc.dram_tensor("wt", [CIN, KH * KW * COUT], mm_dt, kind="ExternalInput").ap()
    out = nc.dram_tensor("out", [BPC, COUT, OH, OW], f32, kind="ExternalOutput").ap()
    WG = KH * KW * 128  # columns per co-half weight chunk

    with tile.TileContext(nc) as tc:
        with (
            tc.tile_pool(name="wpool", bufs=1) as wpool,
            tc.tile_pool(name="scr", bufs=1) as spool,
            tc.tile_pool(name="dpool", bufs=2) as dpool,
            tc.tile_pool(name="opool", bufs=6) as opool,
            tc.tile_pool(name="psum", bufs=7, space="PSUM") as ppool,
            tc.tile_pool(name="wps", bufs=1, space="PSUM") as wppool,
        ):
            # PE warm-up: the HAM clock gate holds the PE at 1.2 GHz until
            # ~3.4us of sustained activity, and the first ~14us here are
            # DMA-bound (preamble + weight/image loads). Run float32r dummy
            # matmuls on scratch data spanning that window so real matmuls
            # start at 2.4 GHz. (fp32 dummies don't work: they lower to
            # LOW_HIGH pairs, take 2-3x longer than budgeted, and the
            # sustained draw downclocks the whole stream.)
            if mm_dtype_name == "float32r":
                wscr = spool.tile([128, 512], f32)
                nc.gpsimd.memset(wscr[:], 0.0)
                wsr = wscr[:].bitcast(mm_dt)
            else:
                wscr = spool.tile([128, 512], mm_dt)
                nc.gpsimd.memset(wscr[:], 0.0)
                wsr = wscr[:]
            wps = wppool.tile([128, 512], f32)
            for _ in range(24):
                nc.tensor.matmul(wps[:], wsr[:, :128], wsr[:], start=True, stop=True)

            wts = []
            for g in range(COUT // 128):
                wtg = wpool.tile([CIN, WG], mm_dt, tag=f"wt{g}")
                wts.append(wtg)
            # first weight half first: the very first matmuls need only this
            nc.sync.dma_start(wts[0][:], w_in[:, :WG])
            dtiles = []
            for n in range(BPC):
                # +2 pad columns: the contiguous N=rows*64 matmul windows
                # read up to 2 elements past the last image row (garbage
                # output columns that are never copied out); fill them with
                # arbitrary real data to keep reads in-bounds and finite.
                chunks = []
                flat = data_in[n].rearrange("c h w -> c (h w)")
                for ci, (c0, crows) in enumerate(CHUNKS):
                    ct = dpool.tile([CIN, crows * W + 2], mm_dt, tag=f"d{ci}")
                    if (c0 + crows) * W + 2 <= H * W:
                        nc.sync.dma_start(
                            ct[:], flat[:, c0 * W : (c0 + crows) * W + 2]
                        )
                    else:
                        nc.sync.dma_start(
                            ct[:, : crows * W], flat[:, c0 * W : (c0 + crows) * W]
                        )
                        nc.sync.dma_start(ct[:, crows * W :], flat[:, :2])
                    chunks.append(ct)
                    if n == 0 and ci == 0:
                        # second weight half is needed only after ~8 groups
                        nc.sync.dma_start(wts[1][:], w_in[:, WG:])
                dtiles.append(chunks)

            for n in range(BPC):
                chunks = dtiles[n]
                for g in range(COUT // 128):
                    for r, (r0, rows) in enumerate(ROW_GROUPS):
                        ci = next(
                            i
                            for i, (c0, crows) in enumerate(CHUNKS)
                            if r0 >= c0 and r0 + rows + KH - 1 <= c0 + crows
                        )
                        ht, hr0 = chunks[ci], r0 - CHUNKS[ci][0]
                        ps = ppool.tile([128, rows * W], f32)
                        for t in range(KH * KW):
                            ky, kx = divmod(t, KW)
                            base = (hr0 + ky) * W + kx
                            nc.tensor.matmul(
                                ps[:],
                                wts[g][:, t * 128 : (t + 1) * 128],
                                ht[:, base : base + rows * W],
                                start=(t == 0),
                                stop=(t == KH * KW - 1),
                            )
                        ot = opool.tile([128, 8 * OW], f32)
                        src = ps[:].rearrange("p (r w) -> p r w", w=W)[:, :, :OW]
                        dst = ot[:, : rows * OW].rearrange(
                            "p (r w) -> p r w", w=OW
                        )
                        if r % 2 == 0:
                            nc.vector.tensor_copy(dst, src)
                        else:
                            nc.scalar.copy(dst, src)
                        nc.scalar.dma_start(
                            out[n].rearrange("c h w -> c (h w)")[
                                g * 128 : (g + 1) * 128, r0 * OW : (r0 + rows) * OW
                            ],
                            ot[:, : rows * OW],
                        )
    nc.compile()
    return nc


def _get_nc(mm_dtype_name="float32r"):
    if mm_dtype_name not in _cache:
        _cache[mm_dtype_name] = build_nc(mm_dtype_name)
    return _cache[mm_dtype_name]


def _np_in_dtype(mm_dtype_name):
    if mm_dtype_name == "bfloat16":
        import ml_dtypes

        return ml_dtypes.bfloat16
    if mm_dtype_name == "float16":
        return np.float16
    return np.float32


def _prep_weights(weights, np_dt):
    # [co, ci, ky, kx] -> [ci][t=ky*3+kx][g][co'] -> [ci][g][t][co'] flat
    w4 = np.asarray(weights, dtype=np.float32).transpose(1, 2, 3, 0)  # ci,ky,kx,co
    w4 = w4.reshape(CIN, KH * KW, COUT // 128, 128).transpose(0, 2, 1, 3)
    return np.ascontiguousarray(w4, dtype=np_dt).reshape(CIN, KH * KW * COUT)


def kernel(data: np.ndarray, weights: np.ndarray, _dtype="float32r") -> np.ndarray:
    from concourse.bass_utils import run_bass_kernel_spmd

    np_dt = _np_in_dtype(_dtype)
    data = np.ascontiguousarray(np.asarray(data), dtype=np_dt)
    wt = _prep_weights(weights, np_dt)

    nc = _get_nc(_dtype)
    in_maps = [
        {"data": data[i * BPC : (i + 1) * BPC], "wt": wt} for i in range(N_CORES)
    ]
    res = run_bass_kernel_spmd(nc, in_maps, core_ids=list(range(N_CORES)))
    return np.concatenate([r["out"] for r in res.results], axis=0)
